# revision 2
# baseline (speedup 1.0000x reference)
"""Causal self-attention with RoPE on 8 NeuronCores.

Sharding: batch (4) x head-group (2 groups of 8 heads) -> 8 shards.
Each core computes attention for one batch element and 8 of the 16 heads,
plus a partial c_proj (rows of w_proj for its heads); the host sums the
two partials per batch element (the 2-way all-reduce of the tensor-parallel
split collapses to a host-side add during unshard).

Self-contained: hardcodes shapes; only needs concourse + numpy + ml_dtypes.
"""

import numpy as np
import ml_dtypes
from contextlib import ExitStack

import concourse.bacc as bacc
import concourse.mybir as mybir
import concourse.tile as tile
from concourse import bass_utils, masks
from concourse.alu_op_type import AluOpType

BF16 = mybir.dt.bfloat16
F32 = mybir.dt.float32

D_MODEL = 1024
N_HEAD = 16
HEAD_DIM = 64
ROPE_THETA = 10000.0
B = 4
T = 2048
N_CORES = 8
H_LOC = 8          # heads per core
C_LOC = H_LOC * HEAD_DIM  # 512 local channels
KC = D_MODEL // 128       # 8 feature chunks
TC = T // 128             # 16 t chunks of 128
NQ = T // 512             # 4 t chunks of 512

_CACHE = {}


def _emit(nc, tc, ctx, aps):
    xT, wqk, wv, wp, cos2, ssign2, mask, out = (
        aps["xT"], aps["wqk"], aps["wv"], aps["wp"],
        aps["cos2"], aps["ssign2"], aps["mask"], aps["out"],
    )
    Exp = mybir.ActivationFunctionType.Exp

    const_pool = ctx.enter_context(tc.tile_pool(name="const", bufs=1))
    in_pool = ctx.enter_context(tc.tile_pool(name="inp", bufs=1))
    qk_pool = ctx.enter_context(tc.tile_pool(name="qk", bufs=1))
    v_pool = ctx.enter_context(tc.tile_pool(name="vp", bufs=1))
    y_pool = ctx.enter_context(tc.tile_pool(name="yp", bufs=1))
    yt_pool = ctx.enter_context(tc.tile_pool(name="ytp", bufs=1))
    tmp_pool = ctx.enter_context(tc.tile_pool(name="tmp", bufs=3))
    att_pool = ctx.enter_context(tc.tile_pool(name="att", bufs=4))
    rec_pool = ctx.enter_context(tc.tile_pool(name="rec", bufs=4))
    out_pool = ctx.enter_context(tc.tile_pool(name="outp", bufs=3))
    ps_mm = ctx.enter_context(tc.tile_pool(name="psmm", bufs=3, space="PSUM"))
    ps_sm = ctx.enter_context(tc.tile_pool(name="pssm", bufs=4, space="PSUM"))

    # ---- constants ----
    cos_sb = const_pool.tile([128, T], BF16, tag="cos")
    nc.sync.dma_start(cos_sb[:], cos2[:])
    ssign_sb = const_pool.tile([128, T], BF16, tag="ssign")
    nc.sync.dma_start(ssign_sb[:], ssign2[:])
    mask_sb = const_pool.tile([128, 128], BF16, tag="mask")
    nc.sync.dma_start(mask_sb[:], mask[:])
    ident = const_pool.tile([128, 128], BF16, tag="ident")
    masks.make_identity(nc, ident[:])

    # ---- input loads ----
    xt = []
    for i in range(KC):
        t = in_pool.tile([128, T], BF16, tag=f"xt{i}", name=f"xt{i}")
        nc.sync.dma_start(t[:], xT[i * 128:(i + 1) * 128, :])
        xt.append(t)
    wqk_sb = []
    for i in range(KC):
        t = in_pool.tile([128, 2 * C_LOC], BF16, tag=f"wqk{i}", name=f"wqk{i}")
        nc.sync.dma_start(t[:], wqk[i * 128:(i + 1) * 128, :])
        wqk_sb.append(t)
    wv_sb = []
    for i in range(KC):
        t = in_pool.tile([128, C_LOC], BF16, tag=f"wv{i}", name=f"wv{i}")
        nc.sync.dma_start(t[:], wv[i * 128:(i + 1) * 128, :])
        wv_sb.append(t)
    wp_sb = []
    for i in range(C_LOC // 128):
        t = in_pool.tile([128, D_MODEL], BF16, tag=f"wp{i}", name=f"wp{i}")
        nc.sync.dma_start(t[:], wp[i * 128:(i + 1) * 128, :])
        wp_sb.append(t)

    # ---- qk^T = (x @ Wqk)^T with RoPE, layout [d, t] (2 heads per tile) ----
    qk_sb = []
    for m in range(8):
        t = qk_pool.tile([128, T], BF16, tag=f"qk{m}", name=f"qk{m}")
        qk_sb.append(t)
    for m in range(8):
        raw = tmp_pool.tile([128, T], BF16, tag="rraw", name="rraw", bufs=2)
        for n in range(NQ):
            ps = ps_mm.tile([128, 512], F32, tag="mm", name="ps_qk")
            for k in range(KC):
                nc.tensor.matmul(
                    ps[:],
                    wqk_sb[k][:, m * 128:(m + 1) * 128],
                    xt[k][:, n * 512:(n + 1) * 512],
                    start=(k == 0), stop=(k == KC - 1),
                )
            nc.scalar.copy(raw[:, n * 512:(n + 1) * 512], ps[:])
        # rotate_half: swap the 32-row blocks within each 64-row head via
        # SBUF->SBUF DMA (vector ops cannot cross partition offsets)
        shuf = tmp_pool.tile([128, T], BF16, tag="rshuf", name="rshuf", bufs=2)
        for blk in range(4):
            p0 = blk * 32
            src = (blk ^ 1) * 32
            nc.sync.dma_start(shuf[p0:p0 + 32, :], raw[src:src + 32, :])
        nc.vector.tensor_tensor(qk_sb[m][:], raw[:], cos_sb[:], op=AluOpType.mult)
        nc.vector.tensor_tensor(shuf[:], shuf[:], ssign_sb[:], op=AluOpType.mult)
        nc.vector.tensor_tensor(qk_sb[m][:], qk_sb[m][:], shuf[:], op=AluOpType.add)

    # ---- v = x @ Wv, natural layout [t, (h, d)] + ones column per head ----
    v_sb = []
    for tcc in range(TC):
        ps = ps_mm.tile([128, 512], F32, tag="mm", name="ps_v")
        for k in range(KC):
            nc.tensor.matmul(
                ps[:],
                xt[k][:, tcc * 128:(tcc + 1) * 128],
                wv_sb[k][:],
                start=(k == 0), stop=(k == KC - 1),
            )
        vt = v_pool.tile([128, H_LOC * 65], BF16, tag=f"v{tcc}", name=f"v{tcc}")
        vv = vt.rearrange("p (h d) -> p h d", d=65)
        nc.vector.tensor_copy(vv[:, :, 0:64], ps.rearrange("p (h d) -> p h d", d=64))
        nc.vector.memset(vv[:, :, 64:65], 1.0)
        v_sb.append(vt)

    # ---- y tiles (natural [t, local_channel]) ----
    y_sb = []
    for tcc in range(TC):
        t = y_pool.tile([128, C_LOC], BF16, tag=f"y{tcc}", name=f"y{tcc}")
        y_sb.append(t)

    # ---- attention: scores^T [tk, tq] -> exp -> att @ v_aug ----
    for h in range(H_LOC):
        qt = qk_sb[h // 2]
        kt = qk_sb[4 + h // 2]
        po = (h % 2) * 64
        for g in range(NQ):
            psy = [
                ps_sm.tile([128, 65], F32, tag="sm", name=f"psy{s}")
                for s in range(4)
            ]
            for j in range(4 * g + 4):
                off = max(0, 128 * j - 512 * g)
                ps_s = ps_mm.tile([128, 512], F32, tag="mm", name="ps_s")
                nc.tensor.matmul(
                    ps_s[:, off:512],
                    kt[po:po + 64, j * 128:(j + 1) * 128],
                    qt[po:po + 64, g * 512 + off:(g + 1) * 512],
                    start=True, stop=True,
                )
                att = att_pool.tile([128, 512], BF16, tag="att", name="att")
                nc.scalar.activation(att[:, off:512], ps_s[:, off:512], Exp,
                                     scale=0.125)
                if j >= 4 * g:
                    # diagonal 128-block: multiplicative causal mask
                    nc.vector.tensor_tensor(
                        att[:, off:off + 128], att[:, off:off + 128],
                        mask_sb[:], op=AluOpType.mult,
                    )
                for sub in range(max(0, j - 4 * g), 4):
                    c = 4 * g + sub
                    nc.tensor.matmul(
                        psy[sub][:],
                        att[:, sub * 128:(sub + 1) * 128],
                        v_sb[j][:, h * 65:(h + 1) * 65],
                        start=(j == 0), stop=(j == c),
                    )
            for sub in range(4):
                c = 4 * g + sub
                rec = rec_pool.tile([128, 1], F32, tag="rec", name="rec")
                nc.vector.reciprocal(rec[:], psy[sub][:, 64:65])
                nc.vector.tensor_scalar_mul(
                    y_sb[c][:, h * 64:(h + 1) * 64],
                    psy[sub][:, 0:64], rec[:],
                )

    # ---- transpose y -> yT [local_channel, t] ----
    yt_sb = []
    for cb in range(C_LOC // 128):
        t = yt_pool.tile([128, T], BF16, tag=f"yt{cb}", name=f"yt{cb}")
        yt_sb.append(t)
    for tcc in range(TC):
        for cb in range(C_LOC // 128):
            pst = ps_mm.tile([128, 128], BF16, tag="mm", name="ps_tr")
            nc.tensor.transpose(
                pst[:], y_sb[tcc][:, cb * 128:(cb + 1) * 128], ident[:]
            )
            nc.scalar.copy(yt_sb[cb][:, tcc * 128:(tcc + 1) * 128], pst[:])

    # ---- partial projection: out = y @ Wp_slice  [t, D_MODEL] f32 ----
    for tcc in range(TC):
        outp = out_pool.tile([128, D_MODEL], F32, tag="out", name="outp")
        for n2 in range(2):
            psp = ps_mm.tile([128, 512], F32, tag="mm", name="ps_p")
            for cb in range(C_LOC // 128):
                nc.tensor.matmul(
                    psp[:],
                    yt_sb[cb][:, tcc * 128:(tcc + 1) * 128],
                    wp_sb[cb][:, n2 * 512:(n2 + 1) * 512],
                    start=(cb == 0), stop=(cb == 3),
                )
            nc.vector.tensor_copy(outp[:, n2 * 512:(n2 + 1) * 512], psp[:])
        nc.sync.dma_start(out[tcc * 128:(tcc + 1) * 128, :], outp[:])


def _build():
    nc = bacc.Bacc("TRN2", debug=False)
    aps = {
        "xT": nc.dram_tensor("xT", [D_MODEL, T], BF16, kind="ExternalInput").ap(),
        "wqk": nc.dram_tensor("wqk", [D_MODEL, 2 * C_LOC], BF16, kind="ExternalInput").ap(),
        "wv": nc.dram_tensor("wv", [D_MODEL, C_LOC], BF16, kind="ExternalInput").ap(),
        "wp": nc.dram_tensor("wp", [C_LOC, D_MODEL], BF16, kind="ExternalInput").ap(),
        "cos2": nc.dram_tensor("cos2", [128, T], BF16, kind="ExternalInput").ap(),
        "ssign2": nc.dram_tensor("ssign2", [128, T], BF16, kind="ExternalInput").ap(),
        "mask": nc.dram_tensor("mask", [128, 128], BF16, kind="ExternalInput").ap(),
        "out": nc.dram_tensor("out", [T, D_MODEL], F32, kind="ExternalOutput").ap(),
    }
    with tile.TileContext(nc) as tc, ExitStack() as ctx:
        _emit(nc, tc, ctx, aps)
    nc.compile()
    return nc


def _rope_tables():
    """cos / sign-folded-sin tables in transposed [d, t] layout, tiled x2
    (two 64-row head patterns per 128-partition tile)."""
    inv_freq = 1.0 / (ROPE_THETA ** (np.arange(0, HEAD_DIM, 2, dtype=np.float32) / HEAD_DIM))
    freqs = np.arange(T, dtype=np.float32)[:, None] * inv_freq[None, :]  # [T, 32]
    emb = np.concatenate([freqs, freqs], axis=-1)  # [T, 64]
    cos = np.cos(emb).T  # [64, T]
    sin = np.sin(emb).T
    ssign = np.concatenate([-sin[:32], sin[32:]], axis=0)  # [64, T]
    cos2 = np.concatenate([cos, cos], axis=0)  # [128, T]
    ssign2 = np.concatenate([ssign, ssign], axis=0)
    bf = ml_dtypes.bfloat16
    return cos2.astype(bf), ssign2.astype(bf)


def _prep_in_maps(x, w_attn, w_proj):
    bf = ml_dtypes.bfloat16
    cos2, ssign2 = _rope_tables()
    i, j = np.indices((128, 128))
    mask01 = (i <= j).astype(bf)  # att^T[tk, tq] valid when tk <= tq

    in_maps = []
    for core in range(N_CORES):
        b, g = divmod(core, 2)
        hsel = slice(g * C_LOC, (g + 1) * C_LOC)
        wq = w_attn[:, 0 * D_MODEL:1 * D_MODEL][:, hsel]
        wk = w_attn[:, 1 * D_MODEL:2 * D_MODEL][:, hsel]
        wv = w_attn[:, 2 * D_MODEL:3 * D_MODEL][:, hsel]
        in_maps.append({
            "xT": np.ascontiguousarray(x[b].T).astype(bf),
            "wqk": np.ascontiguousarray(np.concatenate([wq, wk], axis=1)).astype(bf),
            "wv": np.ascontiguousarray(wv).astype(bf),
            "wp": np.ascontiguousarray(w_proj[hsel, :]).astype(bf),
            "cos2": cos2,
            "ssign2": ssign2,
            "mask": mask01,
        })
    return in_maps


def get_nc():
    if "nc" not in _CACHE:
        _CACHE["nc"] = _build()
    return _CACHE["nc"]


def kernel(x, w_attn, w_proj):
    x = np.asarray(x)
    w_attn = np.asarray(w_attn)
    w_proj = np.asarray(w_proj)
    nc = get_nc()
    in_maps = _prep_in_maps(x, w_attn, w_proj)
    res = bass_utils.run_bass_kernel_spmd(nc, in_maps, core_ids=list(range(N_CORES)))
    out = np.empty((B, T, D_MODEL), dtype=np.float32)
    for b in range(B):
        out[b] = res.results[2 * b]["out"] + res.results[2 * b + 1]["out"]
    return out
